# revision 59
# baseline (speedup 1.0000x reference)
"""Causal attention head (RoPE) kernel for 8 Trainium2 NeuronCores.

Sharding: 8 cores = 2 batches x 4 head-groups (4 heads each), no
cross-device comms. Per core the device works in feature-major layout:

  - host pre-arranges x and the weights c-tile-major so every input DMA is a
    plain contiguous 2D copy (chunked per c-tile so the first projection
    matmul can start ~12us in); Wq/Wk rows are permuted per head so RoPE
    even components land in partitions [0:32) and odd in [32:64) of each
    head's 64-row block.
  - x is shipped in fp8e3 (e3m4: ~1.5% per-element, the dominant input is
    half the bytes so the PE starts ~5us sooner); Q^T/K^T projected in
    512-col chunks over 8 c-tiles; RoPE applied as new = ps*cos -
    swap(ps*sin') with products in bf16 -- the partition swap runs on the PE
    as a permutation matmul trailing the projection chunks by one step, and
    the subtract on DVE.  V is projected directly in natural layout (x
    t-tile stationary, Wv moving) with a ones-column appended per head so
    row 64 of the PV output is the softmax denominator.
  - scores are built transposed (S^T[k,q] = K.Q^T); the two heads of a pair
    write the two 512-col banks of one PSUM tile so a single exp covers both
    heads per 1024 cols (scale 1/32 folded in, no max-subtraction); P^T for
    the pair lives in one SBUF tile (head h at cols [h*w, (h+1)*w)).
  - the PE stream is paced by a work queue: score chunks are the pacing
    stream (exp on Scalar is ~2.5x slower than the score matmuls) and
    between chunks the queue drains single PV-accumulation steps, V tiles,
    and 1/denom broadcasts, so the in-order PE never head-of-line blocks on
    exp and holds its full-speed p-state across pair transitions; the final
    chunk runs a per-head recip + split output writes to shorten the tail.
"""

import os
import sys
from contextlib import ExitStack

import numpy as np

for _p in ("/opt/trn_rl_repo", "/root/.axon_site/_ro/trn_rl_repo"):
    if os.path.isdir(_p) and _p not in sys.path:
        sys.path.append(_p)

import ml_dtypes

import concourse.bass as bass
import concourse.mybir as mybir
import concourse.tile as tile
from concourse import bacc
from concourse.bass_utils import run_bass_kernel_spmd

P = 128
T = 2048
CIN = 1024
NHC = 4          # heads per core
HS = 64
DOUT = NHC * HS  # 256
NCT = CIN // P   # 8 contraction tiles
NTT = T // P     # 16 t/k tiles
SCALE = 1.0 / 32.0  # 1024 ** -0.5

F32 = mybir.dt.float32
BF16 = mybir.dt.bfloat16
FP16 = mybir.dt.float16
FP8 = mybir.dt.float8e3


def _build_nc():
    nc = bacc.Bacc("TRN2")

    xc = nc.dram_tensor("xc", [P, NCT * T], FP8, kind="ExternalInput").ap()
    wq = nc.dram_tensor("wq", [P, NCT * DOUT], BF16, kind="ExternalInput").ap()
    wk = nc.dram_tensor("wk", [P, NCT * DOUT], BF16, kind="ExternalInput").ap()
    wv = nc.dram_tensor("wv", [P, NCT * DOUT], BF16, kind="ExternalInput").ap()
    cos4 = nc.dram_tensor("cos4", [P, T], BF16, kind="ExternalInput").ap()
    sin4 = nc.dram_tensor("sin4", [P, T], BF16, kind="ExternalInput").ap()
    utri = nc.dram_tensor("utri", [P, P], BF16, kind="ExternalInput").ap()
    pswap = nc.dram_tensor("pswap", [P, P], BF16, kind="ExternalInput").ap()
    outT = nc.dram_tensor("outT", [DOUT, T], F32, kind="ExternalOutput").ap()

    with tile.TileContext(nc) as tc, ExitStack() as ctx:
        const_pool = ctx.enter_context(tc.tile_pool(name="const", bufs=1))
        wpool = ctx.enter_context(tc.tile_pool(name="w", bufs=1))
        qkpool = ctx.enter_context(tc.tile_pool(name="qk", bufs=1))
        vpool = ctx.enter_context(tc.tile_pool(name="vaug", bufs=1))
        ptpool = ctx.enter_context(tc.tile_pool(name="pt", bufs=1))
        otpool = ctx.enter_context(tc.tile_pool(name="ot", bufs=4))
        rspool = ctx.enter_context(tc.tile_pool(name="rs", bufs=1))
        phase1 = ExitStack()
        xpool = phase1.enter_context(tc.tile_pool(name="x", bufs=1))
        tmppool = phase1.enter_context(tc.tile_pool(name="tmp", bufs=1))

        # ---- inputs to SBUF.  DMAs are issued in consumption order (queue
        # descriptors drain roughly FIFO across the ring): wq + x c-tiles
        # first, then wk, then the late-needed wv / rope / mask constants.
        w_tiles = {}
        for name, wsrc in (("q", wq), ("k", wk), ("v", wv)):
            w_tiles[name] = wpool.tile(
                [P, NCT * DOUT], BF16, tag=f"w{name}", name=f"w{name}"
            )
        xs = xpool.tile([P, NCT * T], FP8, tag="xs")
        cos_s = const_pool.tile([P, T], BF16, tag="cos")
        sin_s = const_pool.tile([P, T], BF16, tag="sin")
        utri_s = const_pool.tile([P, P], BF16, tag="utri")
        pswap_s = const_pool.tile([P, P], BF16, tag="pswap")
        H = NCT * P  # 1024: one m-half of wq/wk
        dmas = [(w_tiles["q"][:, 0:H], wq[:, 0:H])]
        dmas += [
            (xs[:, c * T:(c + 1) * T], xc[:, c * T:(c + 1) * T]) for c in range(NCT)
        ]
        dmas.insert(3, (w_tiles["k"][:, 0:H], wk[:, 0:H]))
        dmas.insert(5, (w_tiles["q"][:, H:], wq[:, H:]))
        dmas.insert(7, (w_tiles["k"][:, H:], wk[:, H:]))
        dmas += [
            (w_tiles["v"][:], wv), (cos_s[:], cos4), (sin_s[:], sin4),
            (pswap_s[:], pswap), (utri_s[:], utri),
        ]
        _engs = (nc.sync, nc.gpsimd, nc.scalar)
        for i, (dst, src) in enumerate(dmas):
            _engs[i % 3].dma_start(dst, src)
        ones64 = const_pool.tile([1, HS], FP16, tag="ones64")
        nc.vector.memset(ones64[:], 1.0)

        # ---- phase 1a: Q^T/K^T projections + RoPE, 512-col chunks.
        # new = ps*cos - swap(ps*sin'); the partition swap runs on the PE as
        # a permutation matmul on the bf16 sin-product, and the subtract on
        # DVE (one PSUM operand).  Swap matmuls trail the projection chunks
        # by one step so the PE never waits on the DVE multiplies.
        qt = [qkpool.tile([P, T], BF16, tag=f"qt{m}", name=f"qt{m}") for m in range(2)]
        kt = [qkpool.tile([P, T], BF16, tag=f"kt{m}", name=f"kt{m}") for m in range(2)]

        late_rope = []
        with tc.tile_pool(name="pp_proj", bufs=1, space="PSUM") as pp_proj, \
                tc.tile_pool(name="pp_sw", bufs=2, space="PSUM") as pp_sw:
            rq = []

            def flush_swap():
                while rq:
                    dst_sl, apr_p, a_p = rq.pop(0)
                    swp = pp_sw.tile([P, 512], F32, tag="swp")
                    nc.tensor.matmul(
                        swp[:], lhsT=pswap_s[:], rhs=apr_p[:],
                        start=True, stop=True,
                    )
                    nc.vector.tensor_sub(dst_sl, a_p[:], swp[:])

            for ti, (wname, dst, m) in enumerate(
                (("q", qt, 0), ("k", kt, 0), ("q", qt, 1), ("k", kt, 1))
            ):
                w_s = w_tiles[wname]
                if ti == 0:
                    # first projection runs c-outer: one c-tile per 0.85us
                    # matches the x DMA arrival cadence (nch-outer demands
                    # all 8 c-tiles within 1.7us and stalls + drops p-state)
                    ps4 = [
                        pp_proj.tile(
                            [P, 512], F32, tag=f"proj{n}", name=f"proj{n}"
                        )
                        for n in range(4)
                    ]
                    for c in range(NCT):
                        for nch in range(4):
                            nc.tensor.matmul(
                                ps4[nch][:],
                                lhsT=w_s[
                                    :, (m * NCT + c) * P: (m * NCT + c + 1) * P
                                ],
                                rhs=xs[
                                    :, c * T + nch * 512: c * T + (nch + 1) * 512
                                ],
                                start=(c == 0),
                                stop=(c == NCT - 1),
                            )
                    for nch in range(4):
                        sl = slice(nch * 512, (nch + 1) * 512)
                        a = tmppool.tile([P, 512], BF16, tag="ropeA", bufs=4)
                        apr = tmppool.tile([P, 512], BF16, tag="ropeAp", bufs=4)
                        nc.vector.tensor_mul(a[:], ps4[nch][:], cos_s[:, sl])
                        nc.vector.tensor_mul(apr[:], ps4[nch][:], sin_s[:, sl])
                        rq.append((dst[m][:, sl], apr, a))
                    continue
                for nch in range(4):
                    sl = slice(nch * 512, (nch + 1) * 512)
                    ps = pp_proj.tile([P, 512], F32, tag=f"proj{nch}", name="ps")
                    for c in range(NCT):
                        nc.tensor.matmul(
                            ps[:],
                            lhsT=w_s[:, (m * NCT + c) * P: (m * NCT + c + 1) * P],
                            rhs=xs[:, c * T + nch * 512: c * T + (nch + 1) * 512],
                            start=(c == 0),
                            stop=(c == NCT - 1),
                        )
                    a = tmppool.tile([P, 512], BF16, tag="ropeA", bufs=4)
                    apr = tmppool.tile([P, 512], BF16, tag="ropeAp", bufs=4)
                    nc.vector.tensor_mul(a[:], ps[:], cos_s[:, sl])
                    nc.vector.tensor_mul(apr[:], ps[:], sin_s[:, sl])
                    flush_swap()
                    if ti == 3:
                        # k1 is not needed until pair 1 (~45us later): its
                        # swap+sub become queue items so the trailing DVE
                        # chain cannot block pair 0's first scores on the
                        # in-order PE
                        late_rope.append((dst[m][:, sl], apr, a))
                    else:
                        rq.append((dst[m][:, sl], apr, a))
            flush_swap()

        # ---- phase 1b/2 shared machinery
        w_v = w_tiles["v"]
        va = []
        pp_s = ctx.enter_context(tc.tile_pool(name="pp_s", bufs=2, space="PSUM"))
        pp_ob = {}  # pp_o/pp_b created after phase1's PSUM pool closes
        pp_v = phase1.enter_context(tc.tile_pool(name="pp_v", bufs=4, space="PSUM"))

        # Pending PE work items (cost_ns, emit_fn).  Score chunks are the
        # pacing stream (exp on Scalar is ~2.5x slower than the score
        # matmuls); between chunks the queue drains PV / broadcast / V-tile
        # matmuls so the in-order PE stream never head-of-line blocks on exp.
        pending = []

        def pump(budget_ns):
            spent = 0
            while pending and spent < budget_ns:
                cost, fn = pending.pop(0)
                fn()
                spent += cost

        def queue_v_tiles():
            for t in range(NTT):
                def fn(t=t):
                    vt = vpool.tile(
                        [P, NHC * (HS + 1)], BF16, tag=f"vaug{t}", name=f"vaug{t}"
                    )
                    vt_r = vt.rearrange("p (h e) -> p h e", e=HS + 1)
                    nc.gpsimd.memset(vt_r[:, :, HS:HS + 1], 1.0)
                    vp = pp_v.tile([P, DOUT], F32, tag="vp", name="vp")
                    for c in range(NCT):
                        nc.tensor.matmul(
                            vp[:],
                            lhsT=xs[:, c * T + t * P: c * T + (t + 1) * P],
                            rhs=w_v[:, c * DOUT:(c + 1) * DOUT],
                            start=(c == 0),
                            stop=(c == NCT - 1),
                        )
                    nc.vector.tensor_copy(
                        vt_r[:, :, 0:HS], vp.rearrange("p (h d) -> p h d", d=HS)
                    )
                    va.append(vt)
                pending.append((880, fn))

        def ensure_ppob():
            if "o" not in pp_ob:
                # all V-tile items have drained (they precede any PV item in
                # the queue), so xs/tmp/pp_v can be released now
                phase1.close()
                pp_ob["o"] = ctx.enter_context(
                    tc.tile_pool(name="pp_o", bufs=3, space="PSUM")
                )
                pp_ob["b"] = ctx.enter_context(
                    tc.tile_pool(name="pp_b", bufs=1, space="PSUM")
                )

        def scores_j(pair, j, pts):
            """Score matmuls + exp + diag mask for k-block j of a pair,
            pumping queued PE work between chunks."""
            qt_t, kt_t = qt[pair[0] // 2], kt[pair[0] // 2]
            w_j = T - j * P
            ptj = ptpool.tile(
                [P, 2 * w_j], BF16, tag=f"pt{j}", name=f"pt{j}",
                bufs=2 if j < 2 else None,
            )
            pts.append(ptj)
            pt_r = ptj.rearrange("p (h w) -> p h w", h=2)
            for s in range(0, w_j, 512):
                n = min(512, w_j - s)
                ps = pp_s.tile([P, 1024], F32, tag="ps", name="ps")
                for hi in range(2):
                    r0 = hi * HS
                    nc.tensor.matmul(
                        ps[:, hi * 512: hi * 512 + n],
                        lhsT=kt_t[r0:r0 + HS, j * P:(j + 1) * P],
                        rhs=qt_t[r0:r0 + HS, j * P + s: j * P + s + n],
                        start=True,
                        stop=True,
                        tile_position=(r0, 0),
                    )
                nc.scalar.activation(
                    pt_r[:, :, s:s + n],
                    ps.rearrange("p (h c) -> p h c", h=2)[:, :, 0:n],
                    mybir.ActivationFunctionType.Exp,
                    scale=SCALE,
                )
                pump(int(0.9 * n) + 150)
            # causal mask on the diagonal block (col 0 = q-offset j*128)
            for hi in range(2):
                nc.vector.tensor_mul(
                    ptj[:, hi * w_j: hi * w_j + P],
                    ptj[:, hi * w_j: hi * w_j + P],
                    utri_s[:],
                )

        def queue_pv_chunk(pair, qc, pts, norm_q, fin_split=False):
            """Queue the PV accumulation chains for q-chunk qc (both heads),
            in batches of two k-tiles, followed by the denominator recip."""
            q0 = qc * 512
            jmax = 4 * qc + 3
            st = {}
            order = [jj for jj in range(jmax + 1) if jj * P <= q0]
            order += [jj for jj in range(jmax + 1) if jj * P > q0]
            steps = [
                (jj, i == 0, i == jmax) for i, jj in enumerate(order)
            ]
            for hi, h in enumerate(pair):
                for b0 in range(0, len(steps), 1):
                    batch = steps[b0:b0 + 1]
                    def fn(batch=batch, hi=hi, h=h, first=(b0 == 0)):
                        ensure_ppob()
                        if first:
                            st[hi] = pp_ob["o"].tile(
                                [HS + 1, 512], F32, tag="po", name=f"po{hi}"
                            )
                        po = st[hi]
                        for jj, fst, lst in batch:
                            col0 = max(0, jj * P - q0)
                            w_jj = T - jj * P
                            qoff = q0 + col0 - jj * P
                            nc.tensor.matmul(
                                po[:, col0:512],
                                lhsT=va[jj][:, h * (HS + 1):(h + 1) * (HS + 1)],
                                rhs=pts[jj][
                                    :, hi * w_jj + qoff: hi * w_jj + qoff + 512 - col0
                                ],
                                start=fst,
                                stop=lst,
                                skip_group_check=True,
                            )
                            if lst:
                                if fin_split:
                                    # tail chunk: per-head recip+cast so the
                                    # broadcast can start without waiting for
                                    # the other head's chain.  The plain copy
                                    # shifts the denom row to partition 0;
                                    # the custom recip op stays at base 0.
                                    dnh = rspool.tile(
                                        [1, 512], F32, tag=f"dnh{hi}",
                                        name=f"dnh{hi}",
                                    )
                                    nc.vector.tensor_copy(
                                        dnh[:], po[HS:HS + 1, :]
                                    )
                                    rsh = rspool.tile(
                                        [1, 512], F32, tag=f"rsh{hi}",
                                        name=f"rsh{hi}",
                                    )
                                    nc.vector.reciprocal_approx_fast(
                                        rsh[:], dnh[:]
                                    )
                                    r16 = rspool.tile(
                                        [1, 512], FP16, tag=f"rs16_{hi}",
                                        name=f"rs16_{hi}",
                                    )
                                    nc.vector.tensor_copy(r16[:], rsh[:])
                                    st.setdefault("rs16", {})[hi] = r16

                    pending.append((220, fn))

            if not fin_split:
                def fin():
                    # denominators for both heads (rows 0 and 64 keep
                    # partition bases aligned) -> one reciprocal + casts
                    dn = rspool.tile([HS + 1, 512], F32, tag="dn", name="dn")
                    for hi in range(2):
                        nc.vector.tensor_copy(
                            dn[hi * HS:hi * HS + 1, :], st[hi][HS:HS + 1, :]
                        )
                    rs = rspool.tile([HS + 1, 512], F32, tag="rs", name="rs")
                    nc.vector.reciprocal_approx_fast(rs[:], dn[:])
                    st["rs16"] = {}
                    for hi in range(2):
                        r16 = rspool.tile(
                            [1, 512], FP16, tag=f"rs16_{hi}", name=f"rs16_{hi}"
                        )
                        nc.vector.tensor_copy(r16[:], rs[hi * HS:hi * HS + 1, :])
                        st["rs16"][hi] = r16
                pending.append((0, fin))
            norm_q.append((qc, st))

        def queue_norm(pair, item, split=1):
            """Queue the 1/denom broadcast + output write for a chunk."""
            qc, st = item
            q0 = qc * 512
            w = 512 // split
            for hi, h in enumerate(pair):
                for si in range(split):
                    def fn(hi=hi, h=h, si=si):
                        sl = slice(si * w, (si + 1) * w)
                        poS = otpool.tile(
                            [HS, w], F32, tag="poS", name=f"poS{hi}"
                        )
                        nc.scalar.activation(
                            poS[:], st[hi][0:HS, sl],
                            mybir.ActivationFunctionType.Copy,
                        )
                        pb = pp_ob["b"].tile([HS, w], F32, tag="pb", name=f"pb{hi}")
                        nc.tensor.matmul(
                            pb[:],
                            lhsT=ones64[:],
                            rhs=st["rs16"][hi][:, sl],
                            start=True,
                            stop=True,
                        )
                        ot = otpool.tile([HS, w], F32, tag="ot", name="ot")
                        nc.vector.tensor_mul(ot[:], poS[:], pb[:])
                        (nc.sync, nc.gpsimd)[(2 * hi + si) % 2].dma_start(
                            outT[h * HS:(h + 1) * HS, q0 + si * w: q0 + (si + 1) * w],
                            ot[:],
                        )
                    pending.append((400 // split, fn))

        def queue_late_rope():
            for dst_sl, apr_p, a_p in late_rope:
                def fn(dst_sl=dst_sl, apr_p=apr_p, a_p=a_p):
                    swp = pp_s.tile([P, 1024], F32, tag="ps", name="ps")
                    nc.tensor.matmul(
                        swp[:, 0:512], lhsT=pswap_s[:], rhs=apr_p[:],
                        start=True, stop=True,
                    )
                    nc.vector.tensor_sub(dst_sl, a_p[:], swp[:, 0:512])
                pending.append((250, fn))

        # ---- phase 2 schedule
        pairs = ((0, 1), (2, 3))
        queue_v_tiles()
        pending[4:4] = []
        _save = pending[4:]
        del pending[4:]
        queue_late_rope()
        pending.extend(_save)
        for pi, pair in enumerate(pairs):
            pts = []
            norm_q = []
            for j in range(NTT):
                scores_j(pair, j, pts)
                if j % 4 == 0 and j > 0:
                    queue_pv_chunk(pair, j // 4 - 1, pts, norm_q)
                elif j % 4 == 1 and norm_q:
                    queue_norm(pair, norm_q.pop(0))
            queue_pv_chunk(pair, 3, pts, norm_q, fin_split=(pi == 1))
            queue_norm(pair, norm_q.pop(0), split=2 if pi == 1 else 1)
        pump(1 << 30)
    nc.compile()
    return nc


_CACHE = {}


def _get_nc():
    if "nc" not in _CACHE:
        _CACHE["nc"] = _build_nc()
    return _CACHE["nc"]


def _host_inputs(x, Wq, Wk, Wv):
    bf = ml_dtypes.bfloat16
    # RoPE tables (match reference: theta over hs/2 freqs with dim=n_emb)
    i = np.arange(HS // 2, dtype=np.float32)
    theta = np.float32(10000.0) ** (-2.0 * i / np.float32(CIN))
    pos = np.arange(T, dtype=np.float32)
    ang = pos[:, None] * theta[None, :]
    cosT = np.cos(ang).T.astype(np.float32)  # [32, T]
    sinT = np.sin(ang).T.astype(np.float32)
    cos4 = np.ascontiguousarray(np.tile(cosT, (4, 1))).astype(bf)  # [128, T]
    sin4 = np.ascontiguousarray(
        np.tile(np.concatenate([-sinT, sinT], axis=0), (2, 1))
    ).astype(bf)  # rows: [-sin, +sin] x2
    utri_np = np.triu(np.ones((P, P), np.float32)).astype(bf)
    pswap_np = np.zeros((P, P), np.float32)
    pswap_np[np.arange(P), np.arange(P) ^ 32] = 1.0
    pswap_np = pswap_np.astype(bf)

    def cmajor(w):  # [256 out rows, 1024 in] -> [128, 8*256] c-tile-major
        return np.ascontiguousarray(
            w.T.reshape(NCT, P, DOUT).transpose(1, 0, 2).reshape(P, NCT * DOUT)
        ).astype(bf)

    def mmajor(w):  # [256 out, 1024 in] -> [128, m, c, 128] m-half-major
        return np.ascontiguousarray(
            w.T.reshape(NCT, P, 2, P).transpose(1, 2, 0, 3).reshape(P, NCT * DOUT)
        ).astype(bf)

    perm = np.concatenate([np.arange(0, HS, 2), np.arange(1, HS, 2)])
    in_maps = []
    for core in range(8):
        b, g = core // 4, core % 4
        idx = np.concatenate([(4 * g + h) * HS + perm for h in range(NHC)])
        xb = np.ascontiguousarray(
            x[b].T.reshape(NCT, P, T).transpose(1, 0, 2).reshape(P, NCT * T)
        ).astype(ml_dtypes.float8_e3m4)
        m = {
            "xc": xb,
            "wq": mmajor(Wq[idx]),
            "wk": mmajor(Wk[idx]),
            "wv": cmajor(Wv[g * DOUT:(g + 1) * DOUT]),
            "cos4": cos4,
            "sin4": sin4,
            "utri": utri_np,
            "pswap": pswap_np,
        }
        in_maps.append(m)
    return in_maps


def kernel(x, Wq, Wk, Wv, _trace=False, _trace_kwargs=None):
    x = np.asarray(x)
    Wq, Wk, Wv = np.asarray(Wq), np.asarray(Wk), np.asarray(Wv)
    B = x.shape[0]
    nc = _get_nc()
    in_maps = _host_inputs(x, Wq, Wk, Wv)
    res = run_bass_kernel_spmd(
        nc, in_maps, list(range(8)), trace=_trace, **(_trace_kwargs or {})
    )
    out = np.zeros((B, T, CIN), np.float32)
    for core in range(8):
        b, g = core // 4, core % 4
        out[b, :, g * DOUT:(g + 1) * DOUT] = res.results[core]["outT"].T
    if _trace:
        return out, res
    return out


# revision 60
# speedup vs baseline: 1.0056x; 1.0056x over previous
"""Causal attention head (RoPE) kernel for 8 Trainium2 NeuronCores.

Sharding: 8 cores = 2 batches x 4 head-groups (4 heads each), no
cross-device comms. Per core the device works in feature-major layout:

  - host pre-arranges x and the weights c-tile-major so every input DMA is a
    plain contiguous 2D copy (chunked per c-tile so the first projection
    matmul can start ~12us in); Wq/Wk rows are permuted per head so RoPE
    even components land in partitions [0:32) and odd in [32:64) of each
    head's 64-row block.
  - x is shipped in fp8e3 (e3m4: ~1.5% per-element, the dominant input is
    half the bytes so the PE starts ~5us sooner); Q^T/K^T projected in
    512-col chunks over 8 c-tiles; RoPE applied as new = ps*cos -
    swap(ps*sin') with products in bf16 -- the partition swap runs on the PE
    as a permutation matmul trailing the projection chunks by one step, and
    the subtract on DVE.  V is projected directly in natural layout (x
    t-tile stationary, Wv moving) with a ones-column appended per head so
    row 64 of the PV output is the softmax denominator.
  - scores are built transposed (S^T[k,q] = K.Q^T); the two heads of a pair
    write the two 512-col banks of one PSUM tile so a single exp covers both
    heads per 1024 cols (scale 1/32 folded in, no max-subtraction); P^T for
    the pair lives in one SBUF tile (head h at cols [h*w, (h+1)*w)).
  - the PE stream is paced by a work queue: score chunks are the pacing
    stream (exp on Scalar is ~2.5x slower than the score matmuls) and
    between chunks the queue drains single PV-accumulation steps, V tiles,
    and 1/denom broadcasts, so the in-order PE never head-of-line blocks on
    exp and holds its full-speed p-state across pair transitions; the final
    chunk runs a per-head recip + split output writes to shorten the tail.
"""

import os
import sys
from contextlib import ExitStack

import numpy as np

for _p in ("/opt/trn_rl_repo", "/root/.axon_site/_ro/trn_rl_repo"):
    if os.path.isdir(_p) and _p not in sys.path:
        sys.path.append(_p)

import ml_dtypes

import concourse.bass as bass
import concourse.mybir as mybir
import concourse.tile as tile
from concourse import bacc
from concourse.bass_utils import run_bass_kernel_spmd

P = 128
T = 2048
CIN = 1024
NHC = 4          # heads per core
HS = 64
DOUT = NHC * HS  # 256
NCT = CIN // P   # 8 contraction tiles
NTT = T // P     # 16 t/k tiles
SCALE = 1.0 / 32.0  # 1024 ** -0.5

F32 = mybir.dt.float32
BF16 = mybir.dt.bfloat16
FP16 = mybir.dt.float16
FP8 = mybir.dt.float8e3


def _build_nc():
    nc = bacc.Bacc("TRN2")

    xc = nc.dram_tensor("xc", [P, NCT * T], FP8, kind="ExternalInput").ap()
    wq = nc.dram_tensor("wq", [P, NCT * DOUT], BF16, kind="ExternalInput").ap()
    wk = nc.dram_tensor("wk", [P, NCT * DOUT], BF16, kind="ExternalInput").ap()
    wv = nc.dram_tensor("wv", [P, NCT * DOUT], BF16, kind="ExternalInput").ap()
    cos4 = nc.dram_tensor("cos4", [P, T], BF16, kind="ExternalInput").ap()
    sin4 = nc.dram_tensor("sin4", [P, T], BF16, kind="ExternalInput").ap()
    utri = nc.dram_tensor("utri", [P, P], BF16, kind="ExternalInput").ap()
    pswap = nc.dram_tensor("pswap", [P, P], BF16, kind="ExternalInput").ap()
    outT = nc.dram_tensor("outT", [DOUT, T], F32, kind="ExternalOutput").ap()

    with tile.TileContext(nc) as tc, ExitStack() as ctx:
        const_pool = ctx.enter_context(tc.tile_pool(name="const", bufs=1))
        wpool = ctx.enter_context(tc.tile_pool(name="w", bufs=1))
        qkpool = ctx.enter_context(tc.tile_pool(name="qk", bufs=1))
        vpool = ctx.enter_context(tc.tile_pool(name="vaug", bufs=1))
        ptpool = ctx.enter_context(tc.tile_pool(name="pt", bufs=1))
        otpool = ctx.enter_context(tc.tile_pool(name="ot", bufs=4))
        rspool = ctx.enter_context(tc.tile_pool(name="rs", bufs=1))
        phase1 = ExitStack()
        xpool = phase1.enter_context(tc.tile_pool(name="x", bufs=1))
        tmppool = phase1.enter_context(tc.tile_pool(name="tmp", bufs=1))

        # ---- inputs to SBUF.  DMAs are issued in consumption order (queue
        # descriptors drain roughly FIFO across the ring): wq + x c-tiles
        # first, then wk, then the late-needed wv / rope / mask constants.
        w_tiles = {}
        for name, wsrc in (("q", wq), ("k", wk), ("v", wv)):
            w_tiles[name] = wpool.tile(
                [P, NCT * DOUT], BF16, tag=f"w{name}", name=f"w{name}"
            )
        xs = xpool.tile([P, NCT * T], FP8, tag="xs")
        cos_s = const_pool.tile([P, T], BF16, tag="cos")
        sin_s = const_pool.tile([P, T], BF16, tag="sin")
        utri_s = const_pool.tile([P, P], BF16, tag="utri")
        pswap_s = const_pool.tile([P, P], BF16, tag="pswap")
        H = NCT * P  # 1024: one m-half of wq/wk
        dmas = [(w_tiles["q"][:, 0:H], wq[:, 0:H])]
        dmas += [
            (xs[:, c * T:(c + 1) * T], xc[:, c * T:(c + 1) * T]) for c in range(NCT)
        ]
        dmas.insert(3, (w_tiles["k"][:, 0:H], wk[:, 0:H]))
        dmas.insert(5, (w_tiles["q"][:, H:], wq[:, H:]))
        dmas.insert(7, (w_tiles["k"][:, H:], wk[:, H:]))
        dmas += [
            (w_tiles["v"][:], wv), (cos_s[:], cos4), (sin_s[:], sin4),
            (pswap_s[:], pswap), (utri_s[:], utri),
        ]
        _engs = (nc.sync, nc.gpsimd, nc.scalar)
        for i, (dst, src) in enumerate(dmas):
            _engs[i % 3].dma_start(dst, src)
        ones64 = const_pool.tile([1, HS], FP16, tag="ones64")
        nc.vector.memset(ones64[:], 1.0)

        # ---- phase 1a: Q^T/K^T projections + RoPE, 512-col chunks.
        # new = ps*cos - swap(ps*sin'); the partition swap runs on the PE as
        # a permutation matmul on the bf16 sin-product, and the subtract on
        # DVE (one PSUM operand).  Swap matmuls trail the projection chunks
        # by one step so the PE never waits on the DVE multiplies.
        qt = [qkpool.tile([P, T], BF16, tag=f"qt{m}", name=f"qt{m}") for m in range(2)]
        kt = [qkpool.tile([P, T], BF16, tag=f"kt{m}", name=f"kt{m}") for m in range(2)]

        late_rope = []
        with tc.tile_pool(name="pp_proj", bufs=3, space="PSUM") as pp_proj, \
                tc.tile_pool(name="pp_sw", bufs=2, space="PSUM") as pp_sw:
            rq = []

            def flush_swap():
                while rq:
                    dst_sl, apr_p, a_p = rq.pop(0)
                    swp = pp_sw.tile([P, 512], F32, tag="swp")
                    nc.tensor.matmul(
                        swp[:], lhsT=pswap_s[:], rhs=apr_p[:],
                        start=True, stop=True,
                    )
                    nc.vector.tensor_sub(dst_sl, a_p[:], swp[:])

            for ti, (wname, dst, m) in enumerate(
                (("q", qt, 0), ("k", kt, 0), ("q", qt, 1), ("k", kt, 1))
            ):
                w_s = w_tiles[wname]
                for nch in range(4):
                    sl = slice(nch * 512, (nch + 1) * 512)
                    ps = pp_proj.tile([P, 512], F32, tag="proj")
                    for c in range(NCT):
                        nc.tensor.matmul(
                            ps[:],
                            lhsT=w_s[:, (m * NCT + c) * P: (m * NCT + c + 1) * P],
                            rhs=xs[:, c * T + nch * 512: c * T + (nch + 1) * 512],
                            start=(c == 0),
                            stop=(c == NCT - 1),
                        )
                    a = tmppool.tile([P, 512], BF16, tag="ropeA", bufs=4)
                    apr = tmppool.tile([P, 512], BF16, tag="ropeAp", bufs=4)
                    nc.vector.tensor_mul(a[:], ps[:], cos_s[:, sl])
                    nc.vector.tensor_mul(apr[:], ps[:], sin_s[:, sl])
                    flush_swap()
                    if ti == 3:
                        # k1 is not needed until pair 1 (~45us later): its
                        # swap+sub become queue items so the trailing DVE
                        # chain cannot block pair 0's first scores on the
                        # in-order PE
                        late_rope.append((dst[m][:, sl], apr, a))
                    else:
                        rq.append((dst[m][:, sl], apr, a))
            flush_swap()

        # ---- phase 1b/2 shared machinery
        w_v = w_tiles["v"]
        va = []
        pp_s = ctx.enter_context(tc.tile_pool(name="pp_s", bufs=2, space="PSUM"))
        pp_ob = {}  # pp_o/pp_b created after phase1's PSUM pool closes
        pp_v = phase1.enter_context(tc.tile_pool(name="pp_v", bufs=4, space="PSUM"))

        # Pending PE work items (cost_ns, emit_fn).  Score chunks are the
        # pacing stream (exp on Scalar is ~2.5x slower than the score
        # matmuls); between chunks the queue drains PV / broadcast / V-tile
        # matmuls so the in-order PE stream never head-of-line blocks on exp.
        pending = []

        def pump(budget_ns):
            spent = 0
            while pending and spent < budget_ns:
                cost, fn = pending.pop(0)
                fn()
                spent += cost

        def queue_v_tiles():
            for t in range(NTT):
                def fn(t=t):
                    vt = vpool.tile(
                        [P, NHC * (HS + 1)], BF16, tag=f"vaug{t}", name=f"vaug{t}"
                    )
                    vt_r = vt.rearrange("p (h e) -> p h e", e=HS + 1)
                    nc.gpsimd.memset(vt_r[:, :, HS:HS + 1], 1.0)
                    vp = pp_v.tile([P, DOUT], F32, tag="vp", name="vp")
                    for c in range(NCT):
                        nc.tensor.matmul(
                            vp[:],
                            lhsT=xs[:, c * T + t * P: c * T + (t + 1) * P],
                            rhs=w_v[:, c * DOUT:(c + 1) * DOUT],
                            start=(c == 0),
                            stop=(c == NCT - 1),
                        )
                    nc.vector.tensor_copy(
                        vt_r[:, :, 0:HS], vp.rearrange("p (h d) -> p h d", d=HS)
                    )
                    va.append(vt)
                pending.append((880, fn))

        def ensure_ppob():
            if "o" not in pp_ob:
                # all V-tile items have drained (they precede any PV item in
                # the queue), so xs/tmp/pp_v can be released now
                phase1.close()
                pp_ob["o"] = ctx.enter_context(
                    tc.tile_pool(name="pp_o", bufs=3, space="PSUM")
                )
                pp_ob["b"] = ctx.enter_context(
                    tc.tile_pool(name="pp_b", bufs=1, space="PSUM")
                )

        def scores_j(pair, j, pts):
            """Score matmuls + exp + diag mask for k-block j of a pair,
            pumping queued PE work between chunks."""
            qt_t, kt_t = qt[pair[0] // 2], kt[pair[0] // 2]
            w_j = T - j * P
            ptj = ptpool.tile(
                [P, 2 * w_j], BF16, tag=f"pt{j}", name=f"pt{j}",
                bufs=2 if j < 2 else None,
            )
            pts.append(ptj)
            pt_r = ptj.rearrange("p (h w) -> p h w", h=2)
            for s in range(0, w_j, 512):
                n = min(512, w_j - s)
                ps = pp_s.tile([P, 1024], F32, tag="ps", name="ps")
                for hi in range(2):
                    r0 = hi * HS
                    nc.tensor.matmul(
                        ps[:, hi * 512: hi * 512 + n],
                        lhsT=kt_t[r0:r0 + HS, j * P:(j + 1) * P],
                        rhs=qt_t[r0:r0 + HS, j * P + s: j * P + s + n],
                        start=True,
                        stop=True,
                        tile_position=(r0, 0),
                    )
                nc.scalar.activation(
                    pt_r[:, :, s:s + n],
                    ps.rearrange("p (h c) -> p h c", h=2)[:, :, 0:n],
                    mybir.ActivationFunctionType.Exp,
                    scale=SCALE,
                )
                pump(int(0.9 * n) + 150)
            # causal mask on the diagonal block (col 0 = q-offset j*128)
            for hi in range(2):
                nc.vector.tensor_mul(
                    ptj[:, hi * w_j: hi * w_j + P],
                    ptj[:, hi * w_j: hi * w_j + P],
                    utri_s[:],
                )

        def queue_pv_chunk(pair, qc, pts, norm_q, fin_split=False):
            """Queue the PV accumulation chains for q-chunk qc (both heads),
            in batches of two k-tiles, followed by the denominator recip."""
            q0 = qc * 512
            jmax = 4 * qc + 3
            st = {}
            order = [jj for jj in range(jmax + 1) if jj * P <= q0]
            order += [jj for jj in range(jmax + 1) if jj * P > q0]
            steps = [
                (jj, i == 0, i == jmax) for i, jj in enumerate(order)
            ]
            for hi, h in enumerate(pair):
                for b0 in range(0, len(steps), 1):
                    batch = steps[b0:b0 + 1]
                    def fn(batch=batch, hi=hi, h=h, first=(b0 == 0)):
                        ensure_ppob()
                        if first:
                            st[hi] = pp_ob["o"].tile(
                                [HS + 1, 512], F32, tag="po", name=f"po{hi}"
                            )
                        po = st[hi]
                        for jj, fst, lst in batch:
                            col0 = max(0, jj * P - q0)
                            w_jj = T - jj * P
                            qoff = q0 + col0 - jj * P
                            nc.tensor.matmul(
                                po[:, col0:512],
                                lhsT=va[jj][:, h * (HS + 1):(h + 1) * (HS + 1)],
                                rhs=pts[jj][
                                    :, hi * w_jj + qoff: hi * w_jj + qoff + 512 - col0
                                ],
                                start=fst,
                                stop=lst,
                                skip_group_check=True,
                            )
                            if lst:
                                if fin_split:
                                    # tail chunk: per-head recip+cast so the
                                    # broadcast can start without waiting for
                                    # the other head's chain.  The plain copy
                                    # shifts the denom row to partition 0;
                                    # the custom recip op stays at base 0.
                                    dnh = rspool.tile(
                                        [1, 512], F32, tag=f"dnh{hi}",
                                        name=f"dnh{hi}",
                                    )
                                    nc.vector.tensor_copy(
                                        dnh[:], po[HS:HS + 1, :]
                                    )
                                    rsh = rspool.tile(
                                        [1, 512], F32, tag=f"rsh{hi}",
                                        name=f"rsh{hi}",
                                    )
                                    nc.vector.reciprocal_approx_fast(
                                        rsh[:], dnh[:]
                                    )
                                    r16 = rspool.tile(
                                        [1, 512], FP16, tag=f"rs16_{hi}",
                                        name=f"rs16_{hi}",
                                    )
                                    nc.vector.tensor_copy(r16[:], rsh[:])
                                    st.setdefault("rs16", {})[hi] = r16

                    pending.append((220, fn))

            if not fin_split:
                def fin():
                    # denominators for both heads (rows 0 and 64 keep
                    # partition bases aligned) -> one reciprocal + casts
                    dn = rspool.tile([HS + 1, 512], F32, tag="dn", name="dn")
                    for hi in range(2):
                        nc.vector.tensor_copy(
                            dn[hi * HS:hi * HS + 1, :], st[hi][HS:HS + 1, :]
                        )
                    rs = rspool.tile([HS + 1, 512], F32, tag="rs", name="rs")
                    nc.vector.reciprocal_approx_fast(rs[:], dn[:])
                    st["rs16"] = {}
                    for hi in range(2):
                        r16 = rspool.tile(
                            [1, 512], FP16, tag=f"rs16_{hi}", name=f"rs16_{hi}"
                        )
                        nc.vector.tensor_copy(r16[:], rs[hi * HS:hi * HS + 1, :])
                        st["rs16"][hi] = r16
                pending.append((0, fin))
            norm_q.append((qc, st))

        def queue_norm(pair, item, split=1):
            """Queue the 1/denom broadcast + output write for a chunk."""
            qc, st = item
            q0 = qc * 512
            w = 512 // split
            for hi, h in enumerate(pair):
                for si in range(split):
                    def fn(hi=hi, h=h, si=si):
                        sl = slice(si * w, (si + 1) * w)
                        poS = otpool.tile(
                            [HS, w], F32, tag="poS", name=f"poS{hi}"
                        )
                        nc.scalar.activation(
                            poS[:], st[hi][0:HS, sl],
                            mybir.ActivationFunctionType.Copy,
                        )
                        pb = pp_ob["b"].tile([HS, w], F32, tag="pb", name=f"pb{hi}")
                        nc.tensor.matmul(
                            pb[:],
                            lhsT=ones64[:],
                            rhs=st["rs16"][hi][:, sl],
                            start=True,
                            stop=True,
                        )
                        ot = otpool.tile([HS, w], F32, tag="ot", name="ot")
                        nc.vector.tensor_mul(ot[:], poS[:], pb[:])
                        (nc.sync, nc.gpsimd)[(2 * hi + si) % 2].dma_start(
                            outT[h * HS:(h + 1) * HS, q0 + si * w: q0 + (si + 1) * w],
                            ot[:],
                        )
                    pending.append((400 // split, fn))

        def queue_late_rope():
            for dst_sl, apr_p, a_p in late_rope:
                def fn(dst_sl=dst_sl, apr_p=apr_p, a_p=a_p):
                    swp = pp_s.tile([P, 1024], F32, tag="ps", name="ps")
                    nc.tensor.matmul(
                        swp[:, 0:512], lhsT=pswap_s[:], rhs=apr_p[:],
                        start=True, stop=True,
                    )
                    nc.vector.tensor_sub(dst_sl, a_p[:], swp[:, 0:512])
                pending.append((250, fn))

        # ---- phase 2 schedule
        pairs = ((0, 1), (2, 3))
        queue_v_tiles()
        pending[4:4] = []
        _save = pending[4:]
        del pending[4:]
        queue_late_rope()
        pending.extend(_save)
        for pi, pair in enumerate(pairs):
            pts = []
            norm_q = []
            for j in range(NTT):
                scores_j(pair, j, pts)
                if j % 4 == 0 and j > 0:
                    queue_pv_chunk(pair, j // 4 - 1, pts, norm_q)
                elif j % 4 == 1 and norm_q:
                    queue_norm(pair, norm_q.pop(0))
            queue_pv_chunk(pair, 3, pts, norm_q, fin_split=(pi == 1))
            queue_norm(pair, norm_q.pop(0), split=2 if pi == 1 else 1)
        pump(1 << 30)
    nc.compile()
    return nc


_CACHE = {}


def _get_nc():
    if "nc" not in _CACHE:
        _CACHE["nc"] = _build_nc()
    return _CACHE["nc"]


def _host_inputs(x, Wq, Wk, Wv):
    bf = ml_dtypes.bfloat16
    # RoPE tables (match reference: theta over hs/2 freqs with dim=n_emb)
    i = np.arange(HS // 2, dtype=np.float32)
    theta = np.float32(10000.0) ** (-2.0 * i / np.float32(CIN))
    pos = np.arange(T, dtype=np.float32)
    ang = pos[:, None] * theta[None, :]
    cosT = np.cos(ang).T.astype(np.float32)  # [32, T]
    sinT = np.sin(ang).T.astype(np.float32)
    cos4 = np.ascontiguousarray(np.tile(cosT, (4, 1))).astype(bf)  # [128, T]
    sin4 = np.ascontiguousarray(
        np.tile(np.concatenate([-sinT, sinT], axis=0), (2, 1))
    ).astype(bf)  # rows: [-sin, +sin] x2
    utri_np = np.triu(np.ones((P, P), np.float32)).astype(bf)
    pswap_np = np.zeros((P, P), np.float32)
    pswap_np[np.arange(P), np.arange(P) ^ 32] = 1.0
    pswap_np = pswap_np.astype(bf)

    def cmajor(w):  # [256 out rows, 1024 in] -> [128, 8*256] c-tile-major
        return np.ascontiguousarray(
            w.T.reshape(NCT, P, DOUT).transpose(1, 0, 2).reshape(P, NCT * DOUT)
        ).astype(bf)

    def mmajor(w):  # [256 out, 1024 in] -> [128, m, c, 128] m-half-major
        return np.ascontiguousarray(
            w.T.reshape(NCT, P, 2, P).transpose(1, 2, 0, 3).reshape(P, NCT * DOUT)
        ).astype(bf)

    perm = np.concatenate([np.arange(0, HS, 2), np.arange(1, HS, 2)])
    in_maps = []
    for core in range(8):
        b, g = core // 4, core % 4
        idx = np.concatenate([(4 * g + h) * HS + perm for h in range(NHC)])
        xb = np.ascontiguousarray(
            x[b].T.reshape(NCT, P, T).transpose(1, 0, 2).reshape(P, NCT * T)
        ).astype(ml_dtypes.float8_e3m4)
        m = {
            "xc": xb,
            "wq": mmajor(Wq[idx]),
            "wk": mmajor(Wk[idx]),
            "wv": cmajor(Wv[g * DOUT:(g + 1) * DOUT]),
            "cos4": cos4,
            "sin4": sin4,
            "utri": utri_np,
            "pswap": pswap_np,
        }
        in_maps.append(m)
    return in_maps


def kernel(x, Wq, Wk, Wv, _trace=False, _trace_kwargs=None):
    x = np.asarray(x)
    Wq, Wk, Wv = np.asarray(Wq), np.asarray(Wk), np.asarray(Wv)
    B = x.shape[0]
    nc = _get_nc()
    in_maps = _host_inputs(x, Wq, Wk, Wv)
    res = run_bass_kernel_spmd(
        nc, in_maps, list(range(8)), trace=_trace, **(_trace_kwargs or {})
    )
    out = np.zeros((B, T, CIN), np.float32)
    for core in range(8):
        b, g = core // 4, core % 4
        out[b, :, g * DOUT:(g + 1) * DOUT] = res.results[core]["outT"].T
    if _trace:
        return out, res
    return out


# revision 61
# speedup vs baseline: 1.0085x; 1.0029x over previous
"""Causal attention head (RoPE) kernel for 8 Trainium2 NeuronCores.

Sharding: 8 cores = 2 batches x 4 head-groups (4 heads each), no
cross-device comms. Per core the device works in feature-major layout:

  - host pre-arranges x and the weights c-tile-major so every input DMA is a
    plain contiguous 2D copy (chunked per c-tile so the first projection
    matmul can start ~12us in); Wq/Wk rows are permuted per head so RoPE
    even components land in partitions [0:32) and odd in [32:64) of each
    head's 64-row block.
  - x is shipped in fp8e3 (e3m4: ~1.5% per-element, the dominant input is
    half the bytes so the PE starts ~5us sooner); Q^T/K^T projected in
    512-col chunks over 8 c-tiles; RoPE applied as new = ps*cos -
    swap(ps*sin') with products in bf16 -- the partition swap runs on the PE
    as a permutation matmul trailing the projection chunks by one step, and
    the subtract on DVE.  V is projected directly in natural layout (x
    t-tile stationary, Wv moving) with a ones-column appended per head so
    row 64 of the PV output is the softmax denominator.
  - scores are built transposed (S^T[k,q] = K.Q^T); the two heads of a pair
    write the two 512-col banks of one PSUM tile so a single exp covers both
    heads per 1024 cols (scale 1/32 folded in, no max-subtraction); P^T for
    the pair lives in one SBUF tile (head h at cols [h*w, (h+1)*w)).
  - the PE stream is paced by a work queue: score chunks are the pacing
    stream (exp on Scalar is ~2.5x slower than the score matmuls) and
    between chunks the queue drains single PV-accumulation steps, V tiles,
    and 1/denom broadcasts, so the in-order PE never head-of-line blocks on
    exp and holds its full-speed p-state across pair transitions; the final
    chunk runs a per-head recip + split output writes to shorten the tail.
"""

import os
import sys
from contextlib import ExitStack

import numpy as np

for _p in ("/opt/trn_rl_repo", "/root/.axon_site/_ro/trn_rl_repo"):
    if os.path.isdir(_p) and _p not in sys.path:
        sys.path.append(_p)

import ml_dtypes

import concourse.bass as bass
import concourse.mybir as mybir
import concourse.tile as tile
from concourse import bacc
from concourse.bass_utils import run_bass_kernel_spmd

P = 128
T = 2048
CIN = 1024
NHC = 4          # heads per core
HS = 64
DOUT = NHC * HS  # 256
NCT = CIN // P   # 8 contraction tiles
NTT = T // P     # 16 t/k tiles
SCALE = 1.0 / 32.0  # 1024 ** -0.5

F32 = mybir.dt.float32
BF16 = mybir.dt.bfloat16
FP16 = mybir.dt.float16
FP8 = mybir.dt.float8e3


def _build_nc():
    nc = bacc.Bacc("TRN2")

    xc = nc.dram_tensor("xc", [P, NCT * T], FP8, kind="ExternalInput").ap()
    wq = nc.dram_tensor("wq", [P, NCT * DOUT], BF16, kind="ExternalInput").ap()
    wk = nc.dram_tensor("wk", [P, NCT * DOUT], BF16, kind="ExternalInput").ap()
    wv = nc.dram_tensor("wv", [P, NCT * DOUT], BF16, kind="ExternalInput").ap()
    cos4 = nc.dram_tensor("cos4", [P, T], BF16, kind="ExternalInput").ap()
    sin4 = nc.dram_tensor("sin4", [P, T], BF16, kind="ExternalInput").ap()
    utri = nc.dram_tensor("utri", [P, P], BF16, kind="ExternalInput").ap()
    pswap = nc.dram_tensor("pswap", [P, P], BF16, kind="ExternalInput").ap()
    outT = nc.dram_tensor("outT", [DOUT, T], F32, kind="ExternalOutput").ap()

    with tile.TileContext(nc) as tc, ExitStack() as ctx:
        const_pool = ctx.enter_context(tc.tile_pool(name="const", bufs=1))
        wpool = ctx.enter_context(tc.tile_pool(name="w", bufs=1))
        qkpool = ctx.enter_context(tc.tile_pool(name="qk", bufs=1))
        vpool = ctx.enter_context(tc.tile_pool(name="vaug", bufs=1))
        ptpool = ctx.enter_context(tc.tile_pool(name="pt", bufs=1))
        otpool = ctx.enter_context(tc.tile_pool(name="ot", bufs=4))
        rspool = ctx.enter_context(tc.tile_pool(name="rs", bufs=1))
        phase1 = ExitStack()
        xpool = phase1.enter_context(tc.tile_pool(name="x", bufs=1))
        tmppool = phase1.enter_context(tc.tile_pool(name="tmp", bufs=1))

        # ---- inputs to SBUF.  DMAs are issued in consumption order (queue
        # descriptors drain roughly FIFO across the ring): wq + x c-tiles
        # first, then wk, then the late-needed wv / rope / mask constants.
        w_tiles = {}
        for name, wsrc in (("q", wq), ("k", wk), ("v", wv)):
            w_tiles[name] = wpool.tile(
                [P, NCT * DOUT], BF16, tag=f"w{name}", name=f"w{name}"
            )
        xs = xpool.tile([P, NCT * T], FP8, tag="xs")
        cos_s = const_pool.tile([P, T], BF16, tag="cos")
        sin_s = const_pool.tile([P, T], BF16, tag="sin")
        utri_s = const_pool.tile([P, P], BF16, tag="utri")
        pswap_s = const_pool.tile([P, P], BF16, tag="pswap")
        dmas = [(w_tiles["q"][:], wq)]
        dmas += [
            (xs[:, c * T:(c + 1) * T], xc[:, c * T:(c + 1) * T]) for c in range(NCT)
        ]
        dmas.insert(3, (w_tiles["k"][:], wk))
        dmas += [
            (w_tiles["v"][:], wv), (cos_s[:], cos4), (sin_s[:], sin4),
            (pswap_s[:], pswap), (utri_s[:], utri),
        ]
        _engs = (nc.sync, nc.gpsimd, nc.scalar)
        for i, (dst, src) in enumerate(dmas):
            _engs[i % 3].dma_start(dst, src)
        ones64 = const_pool.tile([1, HS], FP16, tag="ones64")
        nc.vector.memset(ones64[:], 1.0)

        # ---- phase 1a: Q^T/K^T projections + RoPE, 512-col chunks.
        # new = ps*cos - swap(ps*sin'); the partition swap runs on the PE as
        # a permutation matmul on the bf16 sin-product, and the subtract on
        # DVE (one PSUM operand).  Swap matmuls trail the projection chunks
        # by one step so the PE never waits on the DVE multiplies.
        qt = [qkpool.tile([P, T], BF16, tag=f"qt{m}", name=f"qt{m}") for m in range(2)]
        kt = [qkpool.tile([P, T], BF16, tag=f"kt{m}", name=f"kt{m}") for m in range(2)]

        late_rope = []
        with tc.tile_pool(name="pp_proj", bufs=3, space="PSUM") as pp_proj, \
                tc.tile_pool(name="pp_sw", bufs=2, space="PSUM") as pp_sw:
            rq = []

            def flush_swap():
                while rq:
                    dst_sl, apr_p, a_p = rq.pop(0)
                    swp = pp_sw.tile([P, 512], F32, tag="swp")
                    nc.tensor.matmul(
                        swp[:], lhsT=pswap_s[:], rhs=apr_p[:],
                        start=True, stop=True,
                    )
                    nc.vector.tensor_sub(dst_sl, a_p[:], swp[:])

            for ti, (wname, dst, m) in enumerate(
                (("q", qt, 0), ("k", kt, 0), ("q", qt, 1), ("k", kt, 1))
            ):
                w_s = w_tiles[wname]
                for nch in range(4):
                    sl = slice(nch * 512, (nch + 1) * 512)
                    ps = pp_proj.tile([P, 512], F32, tag="proj")
                    for c in range(NCT):
                        nc.tensor.matmul(
                            ps[:],
                            lhsT=w_s[:, c * DOUT + m * P: c * DOUT + (m + 1) * P],
                            rhs=xs[:, c * T + nch * 512: c * T + (nch + 1) * 512],
                            start=(c == 0),
                            stop=(c == NCT - 1),
                        )
                    a = tmppool.tile([P, 512], BF16, tag="ropeA", bufs=4)
                    apr = tmppool.tile([P, 512], BF16, tag="ropeAp", bufs=4)
                    nc.vector.tensor_mul(a[:], ps[:], cos_s[:, sl])
                    nc.vector.tensor_mul(apr[:], ps[:], sin_s[:, sl])
                    flush_swap()
                    if ti == 3:
                        # k1 is not needed until pair 1 (~45us later): its
                        # swap+sub become queue items so the trailing DVE
                        # chain cannot block pair 0's first scores on the
                        # in-order PE
                        late_rope.append((dst[m][:, sl], apr, a))
                    else:
                        rq.append((dst[m][:, sl], apr, a))
            flush_swap()

        # ---- phase 1b/2 shared machinery
        w_v = w_tiles["v"]
        va = []
        pp_s = ctx.enter_context(tc.tile_pool(name="pp_s", bufs=2, space="PSUM"))
        pp_ob = {}  # pp_o/pp_b created after phase1's PSUM pool closes
        pp_v = phase1.enter_context(tc.tile_pool(name="pp_v", bufs=4, space="PSUM"))

        # Pending PE work items (cost_ns, emit_fn).  Score chunks are the
        # pacing stream (exp on Scalar is ~2.5x slower than the score
        # matmuls); between chunks the queue drains PV / broadcast / V-tile
        # matmuls so the in-order PE stream never head-of-line blocks on exp.
        pending = []

        def pump(budget_ns):
            spent = 0
            while pending and spent < budget_ns:
                cost, fn = pending.pop(0)
                fn()
                spent += cost

        def queue_v_tiles():
            for t in range(NTT):
                def fn(t=t):
                    vt = vpool.tile(
                        [P, NHC * (HS + 1)], BF16, tag=f"vaug{t}", name=f"vaug{t}"
                    )
                    vt_r = vt.rearrange("p (h e) -> p h e", e=HS + 1)
                    nc.gpsimd.memset(vt_r[:, :, HS:HS + 1], 1.0)
                    vp = pp_v.tile([P, DOUT], F32, tag="vp", name="vp")
                    for c in range(NCT):
                        nc.tensor.matmul(
                            vp[:],
                            lhsT=xs[:, c * T + t * P: c * T + (t + 1) * P],
                            rhs=w_v[:, c * DOUT:(c + 1) * DOUT],
                            start=(c == 0),
                            stop=(c == NCT - 1),
                        )
                    nc.vector.tensor_copy(
                        vt_r[:, :, 0:HS], vp.rearrange("p (h d) -> p h d", d=HS)
                    )
                    va.append(vt)
                pending.append((880, fn))

        def ensure_ppob():
            if "o" not in pp_ob:
                # all V-tile items have drained (they precede any PV item in
                # the queue), so xs/tmp/pp_v can be released now
                phase1.close()
                pp_ob["o"] = ctx.enter_context(
                    tc.tile_pool(name="pp_o", bufs=3, space="PSUM")
                )
                pp_ob["b"] = ctx.enter_context(
                    tc.tile_pool(name="pp_b", bufs=1, space="PSUM")
                )

        def scores_j(pair, j, pts):
            """Score matmuls + exp + diag mask for k-block j of a pair,
            pumping queued PE work between chunks."""
            qt_t, kt_t = qt[pair[0] // 2], kt[pair[0] // 2]
            w_j = T - j * P
            ptj = ptpool.tile(
                [P, 2 * w_j], BF16, tag=f"pt{j}", name=f"pt{j}",
                bufs=2 if j < 2 else None,
            )
            pts.append(ptj)
            pt_r = ptj.rearrange("p (h w) -> p h w", h=2)
            for s in range(0, w_j, 512):
                n = min(512, w_j - s)
                ps = pp_s.tile([P, 1024], F32, tag="ps", name="ps")
                for hi in range(2):
                    r0 = hi * HS
                    nc.tensor.matmul(
                        ps[:, hi * 512: hi * 512 + n],
                        lhsT=kt_t[r0:r0 + HS, j * P:(j + 1) * P],
                        rhs=qt_t[r0:r0 + HS, j * P + s: j * P + s + n],
                        start=True,
                        stop=True,
                        tile_position=(r0, 0),
                    )
                nc.scalar.activation(
                    pt_r[:, :, s:s + n],
                    ps.rearrange("p (h c) -> p h c", h=2)[:, :, 0:n],
                    mybir.ActivationFunctionType.Exp,
                    scale=SCALE,
                )
                pump(int(0.9 * n) + 150)
            # causal mask on the diagonal block (col 0 = q-offset j*128)
            for hi in range(2):
                nc.vector.tensor_mul(
                    ptj[:, hi * w_j: hi * w_j + P],
                    ptj[:, hi * w_j: hi * w_j + P],
                    utri_s[:],
                )

        def queue_pv_chunk(pair, qc, pts, norm_q, fin_split=False):
            """Queue the PV accumulation chains for q-chunk qc (both heads),
            in batches of two k-tiles, followed by the denominator recip."""
            q0 = qc * 512
            jmax = 4 * qc + 3
            st = {}
            order = [jj for jj in range(jmax + 1) if jj * P <= q0]
            order += [jj for jj in range(jmax + 1) if jj * P > q0]
            steps = [
                (jj, i == 0, i == jmax) for i, jj in enumerate(order)
            ]
            for hi, h in enumerate(pair):
                for b0 in range(0, len(steps), 1):
                    batch = steps[b0:b0 + 1]
                    def fn(batch=batch, hi=hi, h=h, first=(b0 == 0)):
                        ensure_ppob()
                        if first:
                            st[hi] = pp_ob["o"].tile(
                                [HS + 1, 512], F32, tag="po", name=f"po{hi}"
                            )
                        po = st[hi]
                        for jj, fst, lst in batch:
                            col0 = max(0, jj * P - q0)
                            w_jj = T - jj * P
                            qoff = q0 + col0 - jj * P
                            nc.tensor.matmul(
                                po[:, col0:512],
                                lhsT=va[jj][:, h * (HS + 1):(h + 1) * (HS + 1)],
                                rhs=pts[jj][
                                    :, hi * w_jj + qoff: hi * w_jj + qoff + 512 - col0
                                ],
                                start=fst,
                                stop=lst,
                                skip_group_check=True,
                            )
                            if lst:
                                if fin_split:
                                    # tail chunk: per-head recip+cast so the
                                    # broadcast can start without waiting for
                                    # the other head's chain.  The plain copy
                                    # shifts the denom row to partition 0;
                                    # the custom recip op stays at base 0.
                                    dnh = rspool.tile(
                                        [1, 512], F32, tag=f"dnh{hi}",
                                        name=f"dnh{hi}",
                                    )
                                    nc.vector.tensor_copy(
                                        dnh[:], po[HS:HS + 1, :]
                                    )
                                    rsh = rspool.tile(
                                        [1, 512], F32, tag=f"rsh{hi}",
                                        name=f"rsh{hi}",
                                    )
                                    nc.vector.reciprocal_approx_fast(
                                        rsh[:], dnh[:]
                                    )
                                    r16 = rspool.tile(
                                        [1, 512], FP16, tag=f"rs16_{hi}",
                                        name=f"rs16_{hi}",
                                    )
                                    nc.vector.tensor_copy(r16[:], rsh[:])
                                    st.setdefault("rs16", {})[hi] = r16

                    pending.append((220, fn))

            if not fin_split:
                def fin():
                    # denominators for both heads (rows 0 and 64 keep
                    # partition bases aligned) -> one reciprocal + casts
                    dn = rspool.tile([HS + 1, 512], F32, tag="dn", name="dn")
                    for hi in range(2):
                        nc.vector.tensor_copy(
                            dn[hi * HS:hi * HS + 1, :], st[hi][HS:HS + 1, :]
                        )
                    rs = rspool.tile([HS + 1, 512], F32, tag="rs", name="rs")
                    nc.vector.reciprocal_approx_fast(rs[:], dn[:])
                    st["rs16"] = {}
                    for hi in range(2):
                        r16 = rspool.tile(
                            [1, 512], FP16, tag=f"rs16_{hi}", name=f"rs16_{hi}"
                        )
                        nc.vector.tensor_copy(r16[:], rs[hi * HS:hi * HS + 1, :])
                        st["rs16"][hi] = r16
                pending.append((0, fin))
            norm_q.append((qc, st))

        def queue_norm(pair, item, split=1):
            """Queue the 1/denom broadcast + output write for a chunk."""
            qc, st = item
            q0 = qc * 512
            w = 512 // split
            for hi, h in enumerate(pair):
                for si in range(split):
                    def fn(hi=hi, h=h, si=si):
                        sl = slice(si * w, (si + 1) * w)
                        poS = otpool.tile(
                            [HS, w], F32, tag="poS", name=f"poS{hi}"
                        )
                        nc.scalar.activation(
                            poS[:], st[hi][0:HS, sl],
                            mybir.ActivationFunctionType.Copy,
                        )
                        pb = pp_ob["b"].tile([HS, w], F32, tag="pb", name=f"pb{hi}")
                        nc.tensor.matmul(
                            pb[:],
                            lhsT=ones64[:],
                            rhs=st["rs16"][hi][:, sl],
                            start=True,
                            stop=True,
                        )
                        ot = otpool.tile([HS, w], F32, tag="ot", name="ot")
                        nc.vector.tensor_mul(ot[:], poS[:], pb[:])
                        (nc.sync, nc.gpsimd)[(2 * hi + si) % 2].dma_start(
                            outT[h * HS:(h + 1) * HS, q0 + si * w: q0 + (si + 1) * w],
                            ot[:],
                        )
                    pending.append((400 // split, fn))

        def queue_late_rope():
            for dst_sl, apr_p, a_p in late_rope:
                def fn(dst_sl=dst_sl, apr_p=apr_p, a_p=a_p):
                    swp = pp_s.tile([P, 1024], F32, tag="ps", name="ps")
                    nc.tensor.matmul(
                        swp[:, 0:512], lhsT=pswap_s[:], rhs=apr_p[:],
                        start=True, stop=True,
                    )
                    nc.vector.tensor_sub(dst_sl, a_p[:], swp[:, 0:512])
                pending.append((250, fn))

        # ---- phase 2 schedule
        pairs = ((0, 1), (2, 3))
        queue_v_tiles()
        pending[4:4] = []
        _save = pending[4:]
        del pending[4:]
        queue_late_rope()
        pending.extend(_save)
        for pi, pair in enumerate(pairs):
            pts = []
            norm_q = []
            for j in range(NTT):
                scores_j(pair, j, pts)
                if j % 4 == 0 and j > 0:
                    queue_pv_chunk(pair, j // 4 - 1, pts, norm_q)
                elif j % 4 == 1 and norm_q:
                    queue_norm(pair, norm_q.pop(0))
            queue_pv_chunk(pair, 3, pts, norm_q, fin_split=(pi == 1))
            queue_norm(pair, norm_q.pop(0), split=2 if pi == 1 else 1)
        pump(1 << 30)
    nc.compile()
    return nc


_CACHE = {}


def _get_nc():
    if "nc" not in _CACHE:
        _CACHE["nc"] = _build_nc()
    return _CACHE["nc"]


def _host_inputs(x, Wq, Wk, Wv):
    bf = ml_dtypes.bfloat16
    # RoPE tables (match reference: theta over hs/2 freqs with dim=n_emb)
    i = np.arange(HS // 2, dtype=np.float32)
    theta = np.float32(10000.0) ** (-2.0 * i / np.float32(CIN))
    pos = np.arange(T, dtype=np.float32)
    ang = pos[:, None] * theta[None, :]
    cosT = np.cos(ang).T.astype(np.float32)  # [32, T]
    sinT = np.sin(ang).T.astype(np.float32)
    cos4 = np.ascontiguousarray(np.tile(cosT, (4, 1))).astype(bf)  # [128, T]
    sin4 = np.ascontiguousarray(
        np.tile(np.concatenate([-sinT, sinT], axis=0), (2, 1))
    ).astype(bf)  # rows: [-sin, +sin] x2
    utri_np = np.triu(np.ones((P, P), np.float32)).astype(bf)
    pswap_np = np.zeros((P, P), np.float32)
    pswap_np[np.arange(P), np.arange(P) ^ 32] = 1.0
    pswap_np = pswap_np.astype(bf)

    def cmajor(w):  # [256 out rows, 1024 in] -> [128, 8*256] c-tile-major
        return np.ascontiguousarray(
            w.T.reshape(NCT, P, DOUT).transpose(1, 0, 2).reshape(P, NCT * DOUT)
        ).astype(bf)


    perm = np.concatenate([np.arange(0, HS, 2), np.arange(1, HS, 2)])
    in_maps = []
    for core in range(8):
        b, g = core // 4, core % 4
        idx = np.concatenate([(4 * g + h) * HS + perm for h in range(NHC)])
        xb = np.ascontiguousarray(
            x[b].T.reshape(NCT, P, T).transpose(1, 0, 2).reshape(P, NCT * T)
        ).astype(ml_dtypes.float8_e3m4)
        m = {
            "xc": xb,
            "wq": cmajor(Wq[idx]),
            "wk": cmajor(Wk[idx]),
            "wv": cmajor(Wv[g * DOUT:(g + 1) * DOUT]),
            "cos4": cos4,
            "sin4": sin4,
            "utri": utri_np,
            "pswap": pswap_np,
        }
        in_maps.append(m)
    return in_maps


def kernel(x, Wq, Wk, Wv, _trace=False, _trace_kwargs=None):
    x = np.asarray(x)
    Wq, Wk, Wv = np.asarray(Wq), np.asarray(Wk), np.asarray(Wv)
    B = x.shape[0]
    nc = _get_nc()
    in_maps = _host_inputs(x, Wq, Wk, Wv)
    res = run_bass_kernel_spmd(
        nc, in_maps, list(range(8)), trace=_trace, **(_trace_kwargs or {})
    )
    out = np.zeros((B, T, CIN), np.float32)
    for core in range(8):
        b, g = core // 4, core % 4
        out[b, :, g * DOUT:(g + 1) * DOUT] = res.results[core]["outT"].T
    if _trace:
        return out, res
    return out


# revision 62
# speedup vs baseline: 1.0144x; 1.0058x over previous
"""Causal attention head (RoPE) kernel for 8 Trainium2 NeuronCores.

Sharding: 8 cores = 2 batches x 4 head-groups (4 heads each), no
cross-device comms. Per core the device works in feature-major layout:

  - host pre-arranges x and the weights c-tile-major so every input DMA is a
    plain contiguous 2D copy (chunked per c-tile so the first projection
    matmul can start ~12us in); Wq/Wk rows are permuted per head so RoPE
    even components land in partitions [0:32) and odd in [32:64) of each
    head's 64-row block.
  - x is shipped in fp8e3 (e3m4: ~1.5% per-element, the dominant input is
    half the bytes so the PE starts ~5us sooner); Q^T/K^T projected in
    512-col chunks over 8 c-tiles; RoPE applied as new = ps*cos -
    swap(ps*sin') with products in bf16 -- the partition swap runs on the PE
    as a permutation matmul trailing the projection chunks by one step, and
    the subtract on DVE.  V is projected directly in natural layout (x
    t-tile stationary, Wv moving) with a ones-column appended per head so
    row 64 of the PV output is the softmax denominator.
  - scores are built transposed (S^T[k,q] = K.Q^T); the two heads of a pair
    write the two 512-col banks of one PSUM tile so a single exp covers both
    heads per 1024 cols (scale 1/32 folded in, no max-subtraction); P^T for
    the pair lives in one SBUF tile (head h at cols [h*w, (h+1)*w)).
  - the PE stream is paced by a work queue: score chunks are the pacing
    stream (exp on Scalar is ~2.5x slower than the score matmuls) and
    between chunks the queue drains single PV-accumulation steps, V tiles,
    and 1/denom broadcasts, so the in-order PE never head-of-line blocks on
    exp and holds its full-speed p-state across pair transitions; the final
    chunk runs a per-head recip + split output writes to shorten the tail.
"""

import os
import sys
from contextlib import ExitStack

import numpy as np

for _p in ("/opt/trn_rl_repo", "/root/.axon_site/_ro/trn_rl_repo"):
    if os.path.isdir(_p) and _p not in sys.path:
        sys.path.append(_p)

import ml_dtypes

import concourse.bass as bass
import concourse.mybir as mybir
import concourse.tile as tile
from concourse import bacc
from concourse.bass_utils import run_bass_kernel_spmd

P = 128
T = 2048
CIN = 1024
NHC = 4          # heads per core
HS = 64
DOUT = NHC * HS  # 256
NCT = CIN // P   # 8 contraction tiles
NTT = T // P     # 16 t/k tiles
SCALE = 1.0 / 32.0  # 1024 ** -0.5

F32 = mybir.dt.float32
BF16 = mybir.dt.bfloat16
FP16 = mybir.dt.float16
FP8 = mybir.dt.float8e3


def _build_nc():
    nc = bacc.Bacc("TRN2")

    xc = nc.dram_tensor("xc", [P, NCT * T], FP8, kind="ExternalInput").ap()
    wq = nc.dram_tensor("wq", [P, NCT * DOUT], BF16, kind="ExternalInput").ap()
    wk = nc.dram_tensor("wk", [P, NCT * DOUT], BF16, kind="ExternalInput").ap()
    wv = nc.dram_tensor("wv", [P, NCT * DOUT], BF16, kind="ExternalInput").ap()
    cos4 = nc.dram_tensor("cos4", [P, T], BF16, kind="ExternalInput").ap()
    sin4 = nc.dram_tensor("sin4", [P, T], BF16, kind="ExternalInput").ap()
    utri = nc.dram_tensor("utri", [P, P], BF16, kind="ExternalInput").ap()
    pswap = nc.dram_tensor("pswap", [P, P], BF16, kind="ExternalInput").ap()
    outT = nc.dram_tensor("outT", [DOUT, T], F32, kind="ExternalOutput").ap()

    with tile.TileContext(nc) as tc, ExitStack() as ctx:
        const_pool = ctx.enter_context(tc.tile_pool(name="const", bufs=1))
        wpool = ctx.enter_context(tc.tile_pool(name="w", bufs=1))
        qkpool = ctx.enter_context(tc.tile_pool(name="qk", bufs=1))
        vpool = ctx.enter_context(tc.tile_pool(name="vaug", bufs=1))
        ptpool = ctx.enter_context(tc.tile_pool(name="pt", bufs=1))
        otpool = ctx.enter_context(tc.tile_pool(name="ot", bufs=6))
        rspool = ctx.enter_context(tc.tile_pool(name="rs", bufs=2))
        phase1 = ExitStack()
        xpool = phase1.enter_context(tc.tile_pool(name="x", bufs=1))
        tmppool = phase1.enter_context(tc.tile_pool(name="tmp", bufs=1))

        # ---- inputs to SBUF.  DMAs are issued in consumption order (queue
        # descriptors drain roughly FIFO across the ring): wq + x c-tiles
        # first, then wk, then the late-needed wv / rope / mask constants.
        w_tiles = {}
        for name, wsrc in (("q", wq), ("k", wk), ("v", wv)):
            w_tiles[name] = wpool.tile(
                [P, NCT * DOUT], BF16, tag=f"w{name}", name=f"w{name}"
            )
        xs = xpool.tile([P, NCT * T], FP8, tag="xs")
        cos_s = const_pool.tile([P, T], BF16, tag="cos")
        sin_s = const_pool.tile([P, T], BF16, tag="sin")
        utri_s = const_pool.tile([P, P], BF16, tag="utri")
        pswap_s = const_pool.tile([P, P], BF16, tag="pswap")
        dmas = [(w_tiles["q"][:], wq)]
        dmas += [
            (xs[:, c * T:(c + 1) * T], xc[:, c * T:(c + 1) * T]) for c in range(NCT)
        ]
        dmas.insert(3, (w_tiles["k"][:], wk))
        dmas += [
            (w_tiles["v"][:], wv), (cos_s[:], cos4), (sin_s[:], sin4),
            (pswap_s[:], pswap), (utri_s[:], utri),
        ]
        _engs = (nc.sync, nc.gpsimd, nc.scalar)
        for i, (dst, src) in enumerate(dmas):
            _engs[i % 3].dma_start(dst, src)
        ones64 = const_pool.tile([1, HS], FP16, tag="ones64")
        nc.vector.memset(ones64[:], 1.0)

        # ---- phase 1a: Q^T/K^T projections + RoPE, 512-col chunks.
        # new = ps*cos - swap(ps*sin'); the partition swap runs on the PE as
        # a permutation matmul on the bf16 sin-product, and the subtract on
        # DVE (one PSUM operand).  Swap matmuls trail the projection chunks
        # by one step so the PE never waits on the DVE multiplies.
        qt = [qkpool.tile([P, T], BF16, tag=f"qt{m}", name=f"qt{m}") for m in range(2)]
        kt = [qkpool.tile([P, T], BF16, tag=f"kt{m}", name=f"kt{m}") for m in range(2)]

        late_rope = []
        with tc.tile_pool(name="pp_proj", bufs=3, space="PSUM") as pp_proj, \
                tc.tile_pool(name="pp_sw", bufs=2, space="PSUM") as pp_sw:
            rq = []

            def flush_swap():
                while rq:
                    dst_sl, apr_p, a_p = rq.pop(0)
                    swp = pp_sw.tile([P, 512], F32, tag="swp")
                    nc.tensor.matmul(
                        swp[:], lhsT=pswap_s[:], rhs=apr_p[:],
                        start=True, stop=True,
                    )
                    nc.vector.tensor_sub(dst_sl, a_p[:], swp[:])

            for ti, (wname, dst, m) in enumerate(
                (("q", qt, 0), ("k", kt, 0), ("q", qt, 1), ("k", kt, 1))
            ):
                w_s = w_tiles[wname]
                for nch in range(4):
                    sl = slice(nch * 512, (nch + 1) * 512)
                    ps = pp_proj.tile([P, 512], F32, tag="proj")
                    for c in range(NCT):
                        nc.tensor.matmul(
                            ps[:],
                            lhsT=w_s[:, c * DOUT + m * P: c * DOUT + (m + 1) * P],
                            rhs=xs[:, c * T + nch * 512: c * T + (nch + 1) * 512],
                            start=(c == 0),
                            stop=(c == NCT - 1),
                        )
                    a = tmppool.tile([P, 512], BF16, tag="ropeA", bufs=4)
                    apr = tmppool.tile([P, 512], BF16, tag="ropeAp", bufs=4)
                    nc.vector.tensor_mul(a[:], ps[:], cos_s[:, sl])
                    nc.vector.tensor_mul(apr[:], ps[:], sin_s[:, sl])
                    flush_swap()
                    if ti == 3:
                        # k1 is not needed until pair 1 (~45us later): its
                        # swap+sub become queue items so the trailing DVE
                        # chain cannot block pair 0's first scores on the
                        # in-order PE
                        late_rope.append((dst[m][:, sl], apr, a))
                    else:
                        rq.append((dst[m][:, sl], apr, a))
            flush_swap()

        # ---- phase 1b/2 shared machinery
        w_v = w_tiles["v"]
        va = []
        pp_s = ctx.enter_context(tc.tile_pool(name="pp_s", bufs=2, space="PSUM"))
        pp_ob = {}  # pp_o/pp_b created after phase1's PSUM pool closes
        pp_v = phase1.enter_context(tc.tile_pool(name="pp_v", bufs=4, space="PSUM"))

        # Pending PE work items (cost_ns, emit_fn).  Score chunks are the
        # pacing stream (exp on Scalar is ~2.5x slower than the score
        # matmuls); between chunks the queue drains PV / broadcast / V-tile
        # matmuls so the in-order PE stream never head-of-line blocks on exp.
        pending = []

        def pump(budget_ns):
            spent = 0
            while pending and spent < budget_ns:
                cost, fn = pending.pop(0)
                fn()
                spent += cost

        def queue_v_tiles():
            for t in range(NTT):
                def fn(t=t):
                    vt = vpool.tile(
                        [P, NHC * (HS + 1)], BF16, tag=f"vaug{t}", name=f"vaug{t}"
                    )
                    vt_r = vt.rearrange("p (h e) -> p h e", e=HS + 1)
                    nc.gpsimd.memset(vt_r[:, :, HS:HS + 1], 1.0)
                    vp = pp_v.tile([P, DOUT], F32, tag="vp", name="vp")
                    for c in range(NCT):
                        nc.tensor.matmul(
                            vp[:],
                            lhsT=xs[:, c * T + t * P: c * T + (t + 1) * P],
                            rhs=w_v[:, c * DOUT:(c + 1) * DOUT],
                            start=(c == 0),
                            stop=(c == NCT - 1),
                        )
                    nc.vector.tensor_copy(
                        vt_r[:, :, 0:HS], vp.rearrange("p (h d) -> p h d", d=HS)
                    )
                    va.append(vt)
                pending.append((880, fn))

        def ensure_ppob():
            if "o" not in pp_ob:
                # all V-tile items have drained (they precede any PV item in
                # the queue), so xs/tmp/pp_v can be released now
                phase1.close()
                pp_ob["o"] = ctx.enter_context(
                    tc.tile_pool(name="pp_o", bufs=3, space="PSUM")
                )
                pp_ob["b"] = ctx.enter_context(
                    tc.tile_pool(name="pp_b", bufs=1, space="PSUM")
                )

        def scores_j(pair, j, pts):
            """Score matmuls + exp + diag mask for k-block j of a pair,
            pumping queued PE work between chunks."""
            qt_t, kt_t = qt[pair[0] // 2], kt[pair[0] // 2]
            w_j = T - j * P
            ptj = ptpool.tile(
                [P, 2 * w_j], BF16, tag=f"pt{j}", name=f"pt{j}",
                bufs=2 if j < 2 else None,
            )
            pts.append(ptj)
            pt_r = ptj.rearrange("p (h w) -> p h w", h=2)
            for s in range(0, w_j, 512):
                n = min(512, w_j - s)
                ps = pp_s.tile([P, 1024], F32, tag="ps", name="ps")
                for hi in range(2):
                    r0 = hi * HS
                    nc.tensor.matmul(
                        ps[:, hi * 512: hi * 512 + n],
                        lhsT=kt_t[r0:r0 + HS, j * P:(j + 1) * P],
                        rhs=qt_t[r0:r0 + HS, j * P + s: j * P + s + n],
                        start=True,
                        stop=True,
                        tile_position=(r0, 0),
                    )
                nc.scalar.activation(
                    pt_r[:, :, s:s + n],
                    ps.rearrange("p (h c) -> p h c", h=2)[:, :, 0:n],
                    mybir.ActivationFunctionType.Exp,
                    scale=SCALE,
                )
                pump(int(0.9 * n) + 150)
            # causal mask on the diagonal block (col 0 = q-offset j*128)
            for hi in range(2):
                nc.vector.tensor_mul(
                    ptj[:, hi * w_j: hi * w_j + P],
                    ptj[:, hi * w_j: hi * w_j + P],
                    utri_s[:],
                )

        def queue_pv_chunk(pair, qc, pts, norm_q, fin_split=False):
            """Queue the PV accumulation chains for q-chunk qc (both heads),
            in batches of two k-tiles, followed by the denominator recip."""
            q0 = qc * 512
            jmax = 4 * qc + 3
            st = {}
            order = [jj for jj in range(jmax + 1) if jj * P <= q0]
            order += [jj for jj in range(jmax + 1) if jj * P > q0]
            steps = [
                (jj, i == 0, i == jmax) for i, jj in enumerate(order)
            ]
            for hi, h in enumerate(pair):
                for b0 in range(0, len(steps), 1):
                    batch = steps[b0:b0 + 1]
                    def fn(batch=batch, hi=hi, h=h, first=(b0 == 0)):
                        ensure_ppob()
                        if first:
                            st[hi] = pp_ob["o"].tile(
                                [HS + 1, 512], F32, tag="po", name=f"po{hi}"
                            )
                        po = st[hi]
                        for jj, fst, lst in batch:
                            col0 = max(0, jj * P - q0)
                            w_jj = T - jj * P
                            qoff = q0 + col0 - jj * P
                            nc.tensor.matmul(
                                po[:, col0:512],
                                lhsT=va[jj][:, h * (HS + 1):(h + 1) * (HS + 1)],
                                rhs=pts[jj][
                                    :, hi * w_jj + qoff: hi * w_jj + qoff + 512 - col0
                                ],
                                start=fst,
                                stop=lst,
                                skip_group_check=True,
                            )
                            if lst:
                                if fin_split:
                                    # tail chunk: per-head recip+cast so the
                                    # broadcast can start without waiting for
                                    # the other head's chain.  The plain copy
                                    # shifts the denom row to partition 0;
                                    # the custom recip op stays at base 0.
                                    dnh = rspool.tile(
                                        [1, 512], F32, tag=f"dnh{hi}",
                                        name=f"dnh{hi}",
                                    )
                                    nc.vector.tensor_copy(
                                        dnh[:], po[HS:HS + 1, :]
                                    )
                                    rsh = rspool.tile(
                                        [1, 512], F32, tag=f"rsh{hi}",
                                        name=f"rsh{hi}",
                                    )
                                    nc.vector.reciprocal_approx_fast(
                                        rsh[:], dnh[:]
                                    )
                                    r16 = rspool.tile(
                                        [1, 512], FP16, tag=f"rs16_{hi}",
                                        name=f"rs16_{hi}",
                                    )
                                    nc.vector.tensor_copy(r16[:], rsh[:])
                                    st.setdefault("rs16", {})[hi] = r16

                    pending.append((220, fn))

            if not fin_split:
                def fin():
                    # denominators for both heads (rows 0 and 64 keep
                    # partition bases aligned) -> one reciprocal + casts
                    dn = rspool.tile([HS + 1, 512], F32, tag="dn", name="dn")
                    for hi in range(2):
                        nc.vector.tensor_copy(
                            dn[hi * HS:hi * HS + 1, :], st[hi][HS:HS + 1, :]
                        )
                    rs = rspool.tile([HS + 1, 512], F32, tag="rs", name="rs")
                    nc.vector.reciprocal_approx_fast(rs[:], dn[:])
                    st["rs16"] = {}
                    for hi in range(2):
                        r16 = rspool.tile(
                            [1, 512], FP16, tag=f"rs16_{hi}", name=f"rs16_{hi}"
                        )
                        nc.vector.tensor_copy(r16[:], rs[hi * HS:hi * HS + 1, :])
                        st["rs16"][hi] = r16
                pending.append((0, fin))
            norm_q.append((qc, st))

        def queue_norm(pair, item, split=1):
            """Queue the 1/denom broadcast + output write for a chunk."""
            qc, st = item
            q0 = qc * 512
            w = 512 // split
            for hi, h in enumerate(pair):
                for si in range(split):
                    def fn(hi=hi, h=h, si=si):
                        sl = slice(si * w, (si + 1) * w)
                        poS = otpool.tile(
                            [HS, w], F32, tag="poS", name=f"poS{hi}"
                        )
                        nc.scalar.activation(
                            poS[:], st[hi][0:HS, sl],
                            mybir.ActivationFunctionType.Copy,
                        )
                        pb = pp_ob["b"].tile([HS, w], F32, tag="pb", name=f"pb{hi}")
                        nc.tensor.matmul(
                            pb[:],
                            lhsT=ones64[:],
                            rhs=st["rs16"][hi][:, sl],
                            start=True,
                            stop=True,
                        )
                        ot = otpool.tile([HS, w], F32, tag="ot", name="ot")
                        nc.vector.tensor_mul(ot[:], poS[:], pb[:])
                        (nc.sync, nc.gpsimd)[(2 * hi + si) % 2].dma_start(
                            outT[h * HS:(h + 1) * HS, q0 + si * w: q0 + (si + 1) * w],
                            ot[:],
                        )
                    pending.append((400 // split, fn))

        def queue_late_rope():
            for dst_sl, apr_p, a_p in late_rope:
                def fn(dst_sl=dst_sl, apr_p=apr_p, a_p=a_p):
                    swp = pp_s.tile([P, 1024], F32, tag="ps", name="ps")
                    nc.tensor.matmul(
                        swp[:, 0:512], lhsT=pswap_s[:], rhs=apr_p[:],
                        start=True, stop=True,
                    )
                    nc.vector.tensor_sub(dst_sl, a_p[:], swp[:, 0:512])
                pending.append((250, fn))

        # ---- phase 2 schedule
        pairs = ((0, 1), (2, 3))
        queue_v_tiles()
        pending[4:4] = []
        _save = pending[4:]
        del pending[4:]
        queue_late_rope()
        pending.extend(_save)
        for pi, pair in enumerate(pairs):
            pts = []
            norm_q = []
            for j in range(NTT):
                scores_j(pair, j, pts)
                if j % 4 == 0 and j > 0:
                    queue_pv_chunk(pair, j // 4 - 1, pts, norm_q)
                elif j % 4 == 1 and norm_q:
                    queue_norm(pair, norm_q.pop(0))
            queue_pv_chunk(pair, 3, pts, norm_q, fin_split=(pi == 1))
            queue_norm(pair, norm_q.pop(0), split=2 if pi == 1 else 1)
        pump(1 << 30)
    nc.compile()
    return nc


_CACHE = {}


def _get_nc():
    if "nc" not in _CACHE:
        _CACHE["nc"] = _build_nc()
    return _CACHE["nc"]


def _host_inputs(x, Wq, Wk, Wv):
    bf = ml_dtypes.bfloat16
    # RoPE tables (match reference: theta over hs/2 freqs with dim=n_emb)
    i = np.arange(HS // 2, dtype=np.float32)
    theta = np.float32(10000.0) ** (-2.0 * i / np.float32(CIN))
    pos = np.arange(T, dtype=np.float32)
    ang = pos[:, None] * theta[None, :]
    cosT = np.cos(ang).T.astype(np.float32)  # [32, T]
    sinT = np.sin(ang).T.astype(np.float32)
    cos4 = np.ascontiguousarray(np.tile(cosT, (4, 1))).astype(bf)  # [128, T]
    sin4 = np.ascontiguousarray(
        np.tile(np.concatenate([-sinT, sinT], axis=0), (2, 1))
    ).astype(bf)  # rows: [-sin, +sin] x2
    utri_np = np.triu(np.ones((P, P), np.float32)).astype(bf)
    pswap_np = np.zeros((P, P), np.float32)
    pswap_np[np.arange(P), np.arange(P) ^ 32] = 1.0
    pswap_np = pswap_np.astype(bf)

    def cmajor(w):  # [256 out rows, 1024 in] -> [128, 8*256] c-tile-major
        return np.ascontiguousarray(
            w.T.reshape(NCT, P, DOUT).transpose(1, 0, 2).reshape(P, NCT * DOUT)
        ).astype(bf)


    perm = np.concatenate([np.arange(0, HS, 2), np.arange(1, HS, 2)])
    in_maps = []
    for core in range(8):
        b, g = core // 4, core % 4
        idx = np.concatenate([(4 * g + h) * HS + perm for h in range(NHC)])
        xb = np.ascontiguousarray(
            x[b].T.reshape(NCT, P, T).transpose(1, 0, 2).reshape(P, NCT * T)
        ).astype(ml_dtypes.float8_e3m4)
        m = {
            "xc": xb,
            "wq": cmajor(Wq[idx]),
            "wk": cmajor(Wk[idx]),
            "wv": cmajor(Wv[g * DOUT:(g + 1) * DOUT]),
            "cos4": cos4,
            "sin4": sin4,
            "utri": utri_np,
            "pswap": pswap_np,
        }
        in_maps.append(m)
    return in_maps


def kernel(x, Wq, Wk, Wv, _trace=False, _trace_kwargs=None):
    x = np.asarray(x)
    Wq, Wk, Wv = np.asarray(Wq), np.asarray(Wk), np.asarray(Wv)
    B = x.shape[0]
    nc = _get_nc()
    in_maps = _host_inputs(x, Wq, Wk, Wv)
    res = run_bass_kernel_spmd(
        nc, in_maps, list(range(8)), trace=_trace, **(_trace_kwargs or {})
    )
    out = np.zeros((B, T, CIN), np.float32)
    for core in range(8):
        b, g = core // 4, core % 4
        out[b, :, g * DOUT:(g + 1) * DOUT] = res.results[core]["outT"].T
    if _trace:
        return out, res
    return out
